# revision 31
# baseline (speedup 1.0000x reference)
"""Trainium2 Bass kernel for gated multi-head attention (B=4, L=1024, D=1024, H=8).

Computation (per the problem's reference):
    q/k/v = x @ Wq/Wk/Wv, split into 8 heads of 128
    score = (q @ k^T) * D**-0.5 ; attn = softmax(score)
    mask_sm = softmax(src_mask)
    attn = (1-sigmoid(g))*attn + sigmoid(g)*mask_sm ; attn /= rowsum(attn)
    out = (attn @ v) @ Wp + bp              -> returns (out, attn)

Since softmax rows each sum to 1, the final renormalization is the identity
in exact arithmetic, so attn = a_l*exp(s) + b_l*exp(m) with per-row scalars
a_l=(1-g)/sum(exp(s)), b_l=g/sum(exp(m)).  Row sums come for free from the
ScalarE activation's accum_out.

Sharding over 8 cores: core c handles batch b=c//2 and head group gg=c%2
(4 of the 8 heads).  The host pre-transposes x (x^T per batch), pre-slices
the weights per head group, and adds the two per-batch output-projection
partial sums at the end (the "all-reduce after the output projection" is a
2-way add done on host, off the device critical path).

Per-core device program (all matmuls N=512 so fp32r runs at 1 cycle/row):
    q^T/k^T = W^T-slices @ x^T, v = x @ Wv    (TensorE, fp32r)
    score tile = q^T_tile^T @ k^T             (TensorE, fp32r)
    e_s = exp(scale*score), e_m = exp(mask)   (ScalarE, accum_out -> row sums)
    u = b_l*(e_s*(a_l/b_l) + e_m)             (DVE scalar_tensor_tensor + GpSimd scale)
    u -> DRAM (attn output), u^T via TensorE transposes (bf16)
    ctx^T = v^T-slices @ u^T                  (TensorE, bf16)
    out_partial = ctx^T-slices^T @ Wp-slice   (TensorE, fp32r)
"""

import numpy as np
import ml_dtypes

import concourse.bacc as bacc
import concourse.mybir as mybir
import concourse.tile as tile
from concourse.bass_utils import run_bass_kernel_spmd
from concourse.masks import make_identity

B, L, D, H = 4, 1024, 1024, 8
P = 128
NHL = H // 2          # heads per core = 4
KT = D // P           # 8 contraction tiles
LT = L // P           # 8 sequence tiles
NW = NHL * P          # 512 = per-core projection width
SCALE = float(D) ** -0.5

F32 = mybir.dt.float32
F32R = mybir.dt.float32r
BF16 = mybir.dt.bfloat16

USE_F32R = True       # fp32r single-pass matmuls (4x faster than fp32)


def _r(ap):
    """Bitcast an fp32 AP to float32r for full-rate PE matmuls."""
    return ap.bitcast(F32R) if USE_F32R else ap


def build_nc():
    nc = bacc.Bacc("TRN2", target_bir_lowering=False, debug=False)
    AX = mybir.ActivationFunctionType

    xT_d = nc.dram_tensor("xT", [D, L], BF16, kind="ExternalInput").ap()
    wq_d = nc.dram_tensor("wq", [D, NW], BF16, kind="ExternalInput").ap()
    wk_d = nc.dram_tensor("wk", [D, NW], BF16, kind="ExternalInput").ap()
    wv_d = nc.dram_tensor("wv", [D, NW], BF16, kind="ExternalInput").ap()
    wp_d = nc.dram_tensor("wp", [NW, D], F32R, kind="ExternalInput").ap()
    mask_d = nc.dram_tensor("mask", [NHL, L, L], F32, kind="ExternalInput").ap()
    g_d = nc.dram_tensor("gvec", [P, NHL], F32, kind="ExternalInput").ap()
    c_d = nc.dram_tensor("cvec", [P, NHL], F32, kind="ExternalInput").ap()
    attn_d = nc.dram_tensor("attn", [NHL, L, L], BF16, kind="ExternalOutput").ap()
    out_d = nc.dram_tensor("outp", [L, D], F32, kind="ExternalOutput").ap()

    with tile.TileContext(nc) as tc:
        with tc.tile_pool(name="persist", bufs=1) as persist:
            qT_sb = persist.tile([P, NHL, L], F32R)     # per-head q^T (dh, l)
            kT_sb = persist.tile([P, NHL, L], F32R)
            v_sb = persist.tile([P, KT, NW], BF16)     # v (t, 4 heads*dh)
            ctxT_sb = persist.tile([P, NHL, L], F32R)   # per-head ctx^T (dh, l)
            wp_sb = persist.tile([P, NHL, D], F32R)
            g_sb = persist.tile([P, NHL], F32)
            c_sb = persist.tile([P, NHL], F32)
            ident = persist.tile([P, P], BF16)

            nc.sync.dma_start(g_sb, g_d)
            nc.sync.dma_start(c_sb, c_d)
            make_identity(nc, ident)

            # ---------------- phase 1: projections ----------------
            with (
                tc.tile_pool(name="xt", bufs=1) as xtp,
                tc.tile_pool(name="wpool", bufs=2) as wpool,
                tc.tile_pool(name="qkv_ps", bufs=4, space="PSUM") as qkv_ps,
            ):
                xT_sb = xtp.tile([P, KT, L], BF16)
                xT_r = xT_d.rearrange("(k p) l -> p k l", p=P)
                wq_sb = wpool.tile([P, KT, NW], F32R, tag="w", name="wq_sb")
                wk_sb = wpool.tile([P, KT, NW], F32R, tag="w", name="wk_sb")
                wq_r = wq_d.rearrange("(k p) n -> p k n", p=P)
                wk_r = wk_d.rearrange("(k p) n -> p k n", p=P)
                # interleaved prologue loads: first-needed chunks first,
                # alternating the SP (sync) and SWDGE (gpsimd) queues
                nc.sync.dma_start(wq_sb[:, 0:2, :], wq_r[:, 0:2, :])
                nc.gpsimd.dma_start(xT_sb[:, 0, :], xT_r[:, 0, :])
                nc.gpsimd.dma_start(xT_sb[:, 1, :], xT_r[:, 1, :])
                nc.sync.dma_start(wq_sb[:, 2:4, :], wq_r[:, 2:4, :])
                nc.gpsimd.dma_start(xT_sb[:, 2, :], xT_r[:, 2, :])
                nc.sync.dma_start(wq_sb[:, 4:8, :], wq_r[:, 4:8, :])
                nc.gpsimd.dma_start(xT_sb[:, 3, :], xT_r[:, 3, :])
                nc.sync.dma_start(xT_sb[:, 4, :], xT_r[:, 4, :])
                nc.gpsimd.dma_start(wk_sb[:, 0:4, :], wk_r[:, 0:4, :])
                nc.sync.dma_start(xT_sb[:, 5, :], xT_r[:, 5, :])
                nc.gpsimd.dma_start(xT_sb[:, 6, :], xT_r[:, 6, :])
                nc.sync.dma_start(wk_sb[:, 4:8, :], wk_r[:, 4:8, :])
                nc.gpsimd.dma_start(xT_sb[:, 7, :], xT_r[:, 7, :])

                for w_sb, dst in ((wq_sb, qT_sb), (wk_sb, kT_sb)):
                    for h in range(NHL):
                        for half in range(2):
                            ps = qkv_ps.tile([P, 512], F32, tag="ps")
                            for k in range(KT):
                                nc.tensor.matmul(
                                    ps,
                                    w_sb[:, k, h * P:(h + 1) * P],
                                    xT_sb[:, k, half * 512:(half + 1) * 512],
                                    start=(k == 0), stop=(k == KT - 1),
                                )
                            nc.scalar.copy(
                                dst[:, h, half * 512:(half + 1) * 512], ps)

                wv_sb = wpool.tile([P, KT, NW], F32R, tag="w", name="wv")
                wv_r = wv_d.rearrange("(k p) n -> p k n", p=P)
                for k in range(0, KT, 2):
                    eng = nc.sync if (k // 2) % 2 == 0 else nc.gpsimd
                    eng.dma_start(wv_sb[:, k:k + 2, :], wv_r[:, k:k + 2, :])
                for j in range(LT):
                    ps = qkv_ps.tile([P, 512], F32, tag="ps")
                    for k in range(KT):
                        nc.tensor.matmul(
                            ps,
                            xT_sb[:, k, j * P:(j + 1) * P],
                            wv_sb[:, k, :],
                            start=(k == 0), stop=(k == KT - 1),
                        )
                    nc.scalar.copy(v_sb[:, j, :], ps)

                wp_r = wp_d.rearrange("(h p) d -> p h d", p=P)
                for hh in range(NHL):
                    eng = nc.sync if hh % 2 == 0 else nc.gpsimd
                    eng.dma_start(wp_sb[:, hh, :], wp_r[:, hh, :])

            # ---------------- phase 2: attention ----------------
            with (
                tc.tile_pool(name="u", bufs=1) as u_pool,
                tc.tile_pool(name="uT", bufs=2) as uT_pool,
                tc.tile_pool(name="streams", bufs=1) as streams,
                tc.tile_pool(name="smalls", bufs=1) as smalls,
                tc.tile_pool(name="score_ps", bufs=2, space="PSUM") as score_ps,
                tc.tile_pool(name="tr_ps", bufs=1, space="PSUM") as tr_ps,
                tc.tile_pool(name="ctx_ps", bufs=1, space="PSUM") as ctx_ps,
            ):
                def emit_transposes(u_t, uT_t, i):
                    for jg in range(2):
                        tp = tr_ps.tile([P, 512], BF16, tag="tr", name="tp")
                        for jj in range(4):
                            j = jg * 4 + jj
                            nc.tensor.transpose(
                                tp[:, jj * P:(jj + 1) * P],
                                u_t[:, i, j * P:(j + 1) * P],
                                ident,
                            )
                        dst = uT_t[:, jg * 4:(jg + 1) * 4, i * P:(i + 1) * P]
                        src = tp.rearrange("p (j l) -> p j l", j=4)
                        nc.vector.tensor_copy(dst, src)

                for h in range(NHL):
                    u_t = u_pool.tile([P, LT, L], BF16, tag="u", name="u")
                    uT_t = uT_pool.tile([P, KT, L], BF16, tag="uT", name="uT")
                    for i in range(LT):
                        mask_t = streams.tile([P, L], F32, tag="mask",
                                              name="mask_t", bufs=5)
                        meng = nc.sync if i % 2 == 0 else nc.gpsimd
                        meng.dma_start(mask_t, mask_d[h, i * P:(i + 1) * P, :])

                        e_m = streams.tile([P, L], F32, tag="em", name="e_m", bufs=5)
                        Sm = smalls.tile([P, 1], F32, tag="Sm", name="Sm", bufs=5)
                        nc.scalar.activation(e_m, mask_t, AX.Exp, accum_out=Sm)

                        sps = score_ps.tile([P, L], F32, tag="sps", name="sps")
                        for half in range(2):
                            nc.tensor.matmul(
                                sps[:, half * 512:(half + 1) * 512],
                                qT_sb[:, h, i * P:(i + 1) * P],
                                kT_sb[:, h, half * 512:(half + 1) * 512],
                                start=True, stop=True,
                            )

                        e_s = streams.tile([P, L], F32, tag="es", name="e_s", bufs=5)
                        Ss = smalls.tile([P, 1], F32, tag="Ss", name="Ss", bufs=5)
                        nc.scalar.activation(e_s, sps, AX.Exp, scale=SCALE,
                                             accum_out=Ss)

                        Rs = smalls.tile([P, 1], F32, tag="Rs", name="Rs", bufs=5)
                        Rm = smalls.tile([P, 1], F32, tag="Rm", name="Rm", bufs=5)
                        t1 = smalls.tile([P, 1], F32, tag="t1", name="t1", bufs=5)
                        ratio = smalls.tile([P, 1], F32, tag="ratio", name="ratio",
                                            bufs=4)
                        bcol = smalls.tile([P, 1], F32, tag="bcol", name="bcol",
                                           bufs=4)
                        nc.vector.reciprocal(Rs, Ss)
                        nc.vector.reciprocal(Rm, Sm)
                        # ratio = ((1-g)/g) * Sm/Ss ; bcol = g/Sm
                        nc.vector.tensor_mul(t1, Sm, Rs)
                        nc.vector.tensor_mul(ratio, t1, c_sb[:, h:h + 1])
                        nc.vector.tensor_mul(bcol, Rm, g_sb[:, h:h + 1])

                        # u = bcol * (e_s*ratio + e_m)
                        nc.vector.scalar_tensor_tensor(
                            e_s, e_s, ratio, e_m,
                            op0=mybir.AluOpType.mult, op1=mybir.AluOpType.add)
                        nc.gpsimd.tensor_scalar_mul(u_t[:, i, :], e_s, bcol)
                        nc.sync.dma_start(attn_d[h, i * P:(i + 1) * P, :],
                                          u_t[:, i, :])
                        if i > 0:
                            emit_transposes(u_t, uT_t, i - 1)
                    emit_transposes(u_t, uT_t, LT - 1)

                if h == 2:
                    # v projection + ctx0 fill PE gaps of the ACT-bound
                    # attention stream (lower priority than head 2/3 scores)
                    for j in range(LT):
                        ps = qkv_ps.tile([P, 512], F32, tag="ps", name="ps")
                        for k in range(KT):
                            nc.tensor.matmul(
                                ps, xT_sb[:, k, j * P:(j + 1) * P],
                                wv_sb[:, k, :],
                                start=(k == 0), stop=(k == KT - 1),
                            )
                        nc.vector.tensor_copy(v_sb[:, j, :], ps)
                    emit_ctx(0, uT_tiles[0])
                elif h == 3:
                    emit_ctx(1, uT_tiles[1])
                    emit_ctx(2, uT_tiles[2])

                    # ctx^T_h = sum_j v_h[j]^T-as-lhsT @ u^T[j]   (bf16)
                    cps = ctx_ps.tile([P, L], F32, tag="ctx", name="cps")
                    for half in range(2):
                        for j in range(KT):
                            nc.tensor.matmul(
                                cps[:, half * 512:(half + 1) * 512],
                                v_sb[:, j, h * P:(h + 1) * P],
                                uT_t[:, j, half * 512:(half + 1) * 512],
                                start=(j == 0), stop=(j == KT - 1),
                            )
                    nc.vector.tensor_copy(ctxT_sb[:, h, 0:512], cps[:, 0:512])
                    nc.vector.tensor_copy(ctxT_sb[:, h, 512:1024], cps[:, 512:1024])

            # ---------------- phase 3: output projection ----------------
            with (
                tc.tile_pool(name="outs", bufs=2) as outs,
                tc.tile_pool(name="op_ps", bufs=2, space="PSUM") as op_ps,
            ):
                for i in range(LT):
                    ps = op_ps.tile([P, D], F32, tag="op", name="ops")
                    for half in range(2):
                        for h in range(NHL):
                            nc.tensor.matmul(
                                ps[:, half * 512:(half + 1) * 512],
                                ctxT_sb[:, h, i * P:(i + 1) * P],
                                wp_sb[:, h, half * 512:(half + 1) * 512],
                                start=(h == 0), stop=(h == NHL - 1),
                            )
                    ob = outs.tile([P, D], F32, tag="ob", name="ob")
                    nc.scalar.copy(ob[:, 0:512], ps[:, 0:512])
                    nc.scalar.copy(ob[:, 512:1024], ps[:, 512:1024])
                    oeng = nc.sync if i % 2 == 0 else nc.gpsimd
                oeng.dma_start(out_d[i * P:(i + 1) * P, :], ob)

    nc.compile()
    return nc


_NC = None


def _get_nc():
    global _NC
    if _NC is None:
        _NC = build_nc()
    return _NC


def make_in_maps(x, src_mask, Wq, Wk, Wv, Wp, gating):
    x = np.asarray(x, np.float32)
    src_mask = np.asarray(src_mask, np.float32).reshape(B, H, L, L)
    Wq = np.asarray(Wq, np.float32)
    Wk = np.asarray(Wk, np.float32)
    Wv = np.asarray(Wv, np.float32)
    Wp = np.asarray(Wp, np.float32)
    g = 1.0 / (1.0 + np.exp(-np.asarray(gating, np.float64)))

    in_maps = []
    for c in range(8):
        b, gg = c // 2, c % 2
        cols = slice(gg * NW, (gg + 1) * NW)
        gloc = g[gg * NHL:(gg + 1) * NHL]
        gv = np.ascontiguousarray(
            np.broadcast_to(gloc.astype(np.float32), (P, NHL)))
        cv = np.ascontiguousarray(
            np.broadcast_to(((1.0 - gloc) / gloc).astype(np.float32), (P, NHL)))
        in_maps.append({
            "xT": np.ascontiguousarray(x[b].T).astype(ml_dtypes.bfloat16),
            "wq": np.ascontiguousarray(Wq[:, cols]).astype(ml_dtypes.bfloat16),
            "wk": np.ascontiguousarray(Wk[:, cols]).astype(ml_dtypes.bfloat16),
            "wv": np.ascontiguousarray(Wv[:, cols]).astype(ml_dtypes.bfloat16),
            "wp": np.ascontiguousarray(Wp[cols, :]),
            "mask": np.ascontiguousarray(src_mask[b, gg * NHL:(gg + 1) * NHL]),
            "gvec": gv,
            "cvec": cv,
        })
    return in_maps


def gather_results(results, bp):
    bp = np.asarray(bp, np.float32)
    out = np.zeros((B, L, D), np.float32)
    attn = np.empty((B, H, L, L), np.float32)
    for c in range(8):
        b, gg = c // 2, c % 2
        out[b] += results[c]["outp"]
        attn[b, gg * NHL:(gg + 1) * NHL] = results[c]["attn"].astype(np.float32)
    out += bp
    return out, attn


def kernel(x, src_mask, Wq, Wk, Wv, Wp, bp, gating):
    nc = _get_nc()
    in_maps = make_in_maps(x, src_mask, Wq, Wk, Wv, Wp, gating)
    res = run_bass_kernel_spmd(nc, in_maps, list(range(8))).results
    return gather_results(res, bp)            def emit_transposes(u_t, uT_t, i):
                    for jg in range(2):
                        tp = tr_ps.tile([P, 512], BF16, tag="tr", name="tp")
                        for jj in range(4):
                            j = jg * 4 + jj
                            nc.tensor.transpose(
                                tp[:, jj * P:(jj + 1) * P],
                                u_t[:, i, j * P:(j + 1) * P],
                                ident,
                            )
                        dst = uT_t[:, jg * 4:(jg + 1) * 4, i * P:(i + 1) * P]
                        src = tp.rearrange("p (j l) -> p j l", j=4)
                        nc.vector.tensor_copy(dst, src)

                for h in range(NHL):
                    u_t = u_pool.tile([P, LT, L], BF16, tag="u", name="u")
                    uT_t = uT_pool.tile([P, KT, L], BF16, tag="uT", name="uT")
                    for i in range(LT):
                        mask_t = streams.tile([P, L], F32, tag="mask",
                                              name="mask_t", bufs=5)
                        meng = nc.sync if i % 2 == 0 else nc.gpsimd
                        meng.dma_start(mask_t, mask_d[h, i * P:(i + 1) * P, :])

                        e_m = streams.tile([P, L], F32, tag="em", name="e_m", bufs=5)
                        Sm = smalls.tile([P, 1], F32, tag="Sm", name="Sm", bufs=5)
                        nc.scalar.activation(e_m, mask_t, AX.Exp, accum_out=Sm)

                        sps = score_ps.tile([P, L], F32, tag="sps", name="sps")
                        for half in range(2):
                            nc.tensor.matmul(
                                sps[:, half * 512:(half + 1) * 512],
                                qT_sb[:, h, i * P:(i + 1) * P],
                                kT_sb[:, h, half * 512:(half + 1) * 512],
                                start=True, stop=True,
                            )

                        e_s = streams.tile([P, L], F32, tag="es", name="e_s", bufs=5)
                        Ss = smalls.tile([P, 1], F32, tag="Ss", name="Ss", bufs=5)
                        nc.scalar.activation(e_s, sps, AX.Exp, scale=SCALE,
                                             accum_out=Ss)

                        Rs = smalls.tile([P, 1], F32, tag="Rs", name="Rs", bufs=5)
                        Rm = smalls.tile([P, 1], F32, tag="Rm", name="Rm", bufs=5)
                        t1 = smalls.tile([P, 1], F32, tag="t1", name="t1", bufs=5)
                        ratio = smalls.tile([P, 1], F32, tag="ratio", name="ratio",
                                            bufs=4)
                        bcol = smalls.tile([P, 1], F32, tag="bcol", name="bcol",
                                           bufs=4)
                        nc.vector.reciprocal(Rs, Ss)
                        nc.vector.reciprocal(Rm, Sm)
                        # ratio = ((1-g)/g) * Sm/Ss ; bcol = g/Sm
                        nc.vector.tensor_mul(t1, Sm, Rs)
                        nc.vector.tensor_mul(ratio, t1, c_sb[:, h:h + 1])
                        nc.vector.tensor_mul(bcol, Rm, g_sb[:, h:h + 1])

                        # u = bcol * (e_s*ratio + e_m)
                        nc.vector.scalar_tensor_tensor(
                            e_s, e_s, ratio, e_m,
                            op0=mybir.AluOpType.mult, op1=mybir.AluOpType.add)
                        nc.gpsimd.tensor_scalar_mul(u_t[:, i, :], e_s, bcol)
                        nc.sync.dma_start(attn_d[h, i * P:(i + 1) * P, :],
                                          u_t[:, i, :])
                        if i > 0:
                            emit_transposes(u_t, uT_t, i - 1)
                    emit_transposes(u_t, uT_t, LT - 1)

                if h == 2:
                    # v projection + ctx0 fill PE gaps of the ACT-bound
                    # attention stream (lower priority than head 2/3 scores)
                    for j in range(LT):
                        ps = qkv_ps.tile([P, 512], F32, tag="ps", name="ps")
                        for k in range(KT):
                            nc.tensor.matmul(
                                ps, xT_sb[:, k, j * P:(j + 1) * P],
                                wv_sb[:, k, :],
                                start=(k == 0), stop=(k == KT - 1),
                            )
                        nc.vector.tensor_copy(v_sb[:, j, :], ps)
                    emit_ctx(0, uT_tiles[0])
                elif h == 3:
                    emit_ctx(1, uT_tiles[1])
                    emit_ctx(2, uT_tiles[2])

                    # ctx^T_h = sum_j v_h[j]^T-as-lhsT @ u^T[j]   (bf16)
                    cps = ctx_ps.tile([P, L], F32, tag="ctx", name="cps")
                    for half in range(2):
                        for j in range(KT):
                            nc.tensor.matmul(
                                cps[:, half * 512:(half + 1) * 512],
                                v_sb[:, j, h * P:(h + 1) * P],
                                uT_t[:, j, half * 512:(half + 1) * 512],
                                start=(j == 0), stop=(j == KT - 1),
                            )
                    nc.vector.tensor_copy(ctxT_sb[:, h, 0:512], cps[:, 0:512])
                    nc.vector.tensor_copy(ctxT_sb[:, h, 512:1024], cps[:, 512:1024])

            # ---------------- phase 3: output projection ----------------
            with (
                tc.tile_pool(name="outs", bufs=2) as outs,
                tc.tile_pool(name="op_ps", bufs=2, space="PSUM") as op_ps,
            ):
                for i in range(LT):
                    ps = op_ps.tile([P, D], F32, tag="op", name="ops")
                    for half in range(2):
                        for h in range(NHL):
                            nc.tensor.matmul(
                                ps[:, half * 512:(half + 1) * 512],
                                ctxT_sb[:, h, i * P:(i + 1) * P],
                                wp_sb[:, h, half * 512:(half + 1) * 512],
                                start=(h == 0), stop=(h == NHL - 1),
                            )
                    ob = outs.tile([P, D], F32, tag="ob", name="ob")
                    nc.scalar.copy(ob[:, 0:512], ps[:, 0:512])
                    nc.scalar.copy(ob[:, 512:1024], ps[:, 512:1024])
                    oeng = nc.sync if i % 2 == 0 else nc.gpsimd
                oeng.dma_start(out_d[i * P:(i + 1) * P, :], ob)

    nc.compile()
    return nc


_NC = None


def _get_nc():
    global _NC
    if _NC is None:
        _NC = build_nc()
    return _NC


def make_in_maps(x, src_mask, Wq, Wk, Wv, Wp, gating):
    x = np.asarray(x, np.float32)
    src_mask = np.asarray(src_mask, np.float32).reshape(B, H, L, L)
    Wq = np.asarray(Wq, np.float32)
    Wk = np.asarray(Wk, np.float32)
    Wv = np.asarray(Wv, np.float32)
    Wp = np.asarray(Wp, np.float32)
    g = 1.0 / (1.0 + np.exp(-np.asarray(gating, np.float64)))

    in_maps = []
    for c in range(8):
        b, gg = c // 2, c % 2
        cols = slice(gg * NW, (gg + 1) * NW)
        gloc = g[gg * NHL:(gg + 1) * NHL]
        gv = np.ascontiguousarray(
            np.broadcast_to(gloc.astype(np.float32), (P, NHL)))
        cv = np.ascontiguousarray(
            np.broadcast_to(((1.0 - gloc) / gloc).astype(np.float32), (P, NHL)))
        in_maps.append({
            "xT": np.ascontiguousarray(x[b].T).astype(ml_dtypes.bfloat16),
            "wq": np.ascontiguousarray(Wq[:, cols]).astype(ml_dtypes.bfloat16),
            "wk": np.ascontiguousarray(Wk[:, cols]).astype(ml_dtypes.bfloat16),
            "wv": np.ascontiguousarray(Wv[:, cols]).astype(ml_dtypes.bfloat16),
            "wp": np.ascontiguousarray(Wp[cols, :]),
            "mask": np.ascontiguousarray(src_mask[b, gg * NHL:(gg + 1) * NHL]),
            "gvec": gv,
            "cvec": cv,
        })
    return in_maps


def gather_results(results, bp):
    bp = np.asarray(bp, np.float32)
    out = np.zeros((B, L, D), np.float32)
    attn = np.empty((B, H, L, L), np.float32)
    for c in range(8):
        b, gg = c // 2, c % 2
        out[b] += results[c]["outp"]
        attn[b, gg * NHL:(gg + 1) * NHL] = results[c]["attn"].astype(np.float32)
    out += bp
    return out, attn


def kernel(x, src_mask, Wq, Wk, Wv, Wp, bp, gating):
    nc = _get_nc()
    in_maps = make_in_maps(x, src_mask, Wq, Wk, Wv, Wp, gating)
    res = run_bass_kernel_spmd(nc, in_maps, list(range(8))).results
    return gather_results(res, bp)
